# revision 1
# baseline (speedup 1.0000x reference)
"""Trainium2 Bass kernel for nn_CrossAttentionLayer.

Reference computation (per batch element b):
    q = x @ Wq            [N, INNER]   (heads: INNER = H*Dh)
    k = ctx @ Wk          [J, INNER]
    v = ctx @ Wv          [J, INNER]
    sim = q_h @ k_h.T * scale   per head -> softmax over J -> @ v_h
    out = concat_heads @ Wo + bo

Sharding: batch (B=8) across 8 cores, one batch element per core, weights
replicated.  No collectives needed.

Per-core plan (all matmuls bf16 operands, fp32 PSUM accumulation):
  - transpose x -> xT [QD, N], ctx -> ctxT [CD, J] via PE transposes
  - QT [INNER, N] = Wq.T @ xT   (stationary Wq chunks, moving xT)
  - KT [INNER, J] = Wk.T @ ctxT
  - V  [J, INNER] = ctxT.T @ Wv, stored padded per head with a ones column
  - per head h: S^T [J, N] = KT_h.T-contract-d @ QT_h  (K=64 contraction)
      P^T = exp(scale * S^T)  on ACT, written bf16
      O[n, 64+1] = sum_jc P^T_jc.T @ Vpad_h_jc   (ones col -> softmax denom)
      evict O unnormalized (ACT), denom col to den buffer
  - normalize per n-tile: rden = 1/den (DVE), O *= rden (broadcast mul)
  - transpose O -> OT [INNER, N]; out = OT.T @ Wo + bo -> DMA out
"""

import sys

if "/opt/trn_rl_repo" not in sys.path:
    sys.path.insert(0, "/opt/trn_rl_repo")

import numpy as np

import concourse.bass as bass
import concourse.mybir as mybir
import concourse.bacc as bacc
import concourse.tile as tile
from concourse import bass_utils
from concourse.masks import make_identity

P = 128
B, N, J = 8, 2048, 1024
QD, CD, H, Dh = 1024, 768, 16, 64
INNER = H * Dh
NT = N // P      # 16 n tiles
JC = J // P      # 8 context chunks
QC = QD // P     # 8 x-feature chunks
CC = CD // P     # 6 ctx-feature chunks
IC = INNER // P  # 8 inner chunks
NBW = 512        # moving-operand block width
NB = N // NBW    # 4
SCALE = float(Dh) ** -0.5

F32 = mybir.dt.float32
BF16 = mybir.dt.bfloat16
EXP = mybir.ActivationFunctionType.Exp

_CACHE = {}


def _build_module():
    nc = bacc.Bacc("TRN2", target_bir_lowering=False, debug=False)

    x_d = nc.dram_tensor("x", [N, QD], F32, kind="ExternalInput")
    ctx_d = nc.dram_tensor("context", [J, CD], F32, kind="ExternalInput")
    wq_d = nc.dram_tensor("Wq", [QD, INNER], F32, kind="ExternalInput")
    wk_d = nc.dram_tensor("Wk", [CD, INNER], F32, kind="ExternalInput")
    wv_d = nc.dram_tensor("Wv", [CD, INNER], F32, kind="ExternalInput")
    wo_d = nc.dram_tensor("Wo", [INNER, QD], F32, kind="ExternalInput")
    bo_d = nc.dram_tensor("bo", [QD], F32, kind="ExternalInput")
    out_d = nc.dram_tensor("out", [N, QD], F32, kind="ExternalOutput")

    with tile.TileContext(nc) as tc:
        _emit(nc, tc, x_d, ctx_d, wq_d, wk_d, wv_d, wo_d, bo_d, out_d)

    nc.compile()
    return nc


def _emit(nc, tc, x_d, ctx_d, wq_d, wk_d, wv_d, wo_d, bo_d, out_d):
    from contextlib import ExitStack

    est = ExitStack()
    with est:
        # ---------- constants ----------
        const = est.enter_context(tc.tile_pool(name="const", bufs=1))
        ones_row = const.tile([1, P], F32, name="ones_row")
        nc.vector.memset(ones_row[:], 1.0)
        ones_bf = const.tile([1, P], BF16, name="ones_bf")
        nc.vector.memset(ones_bf[:], 1.0)
        bo_sb = const.tile([1, QD], F32, name="bo_sb")
        nc.sync.dma_start(bo_sb[:], bo_d[:].unsqueeze(0))
        bias_bc = const.tile([P, QD], BF16, name="bias_bc")

        with tc.tile_pool(name="cpsum", bufs=2, space="PSUM") as cpsum:
            for qb in range(QD // NBW):
                bp = cpsum.tile([P, NBW], F32, name="bp", tag="bp")
                nc.tensor.matmul(
                    bp[:], ones_row[:, :], bo_sb[:, qb * NBW:(qb + 1) * NBW],
                    start=True, stop=True,
                )
                nc.vector.tensor_copy(bias_bc[:, qb * NBW:(qb + 1) * NBW], bp[:])

        # ---------- bf16 casts staged in DRAM (flat = 1 descriptor each) ----
        dram = est.enter_context(tc.tile_pool(name="dram", bufs=1, space="DRAM"))
        x_bf = dram.tile([N, QD], BF16, name="x_bf")
        ctx_bf = dram.tile([J, CD], BF16, name="ctx_bf")
        wq_bf = dram.tile([QD, INNER], BF16, name="wq_bf")
        wk_bf = dram.tile([CD, INNER], BF16, name="wk_bf")
        wv_bf = dram.tile([CD, INNER], BF16, name="wv_bf")
        wo_bf = dram.tile([INNER, QD], BF16, name="wo_bf")
        nc.gpsimd.dma_start(ctx_bf[:].flatten(), ctx_d[:].flatten())
        nc.gpsimd.dma_start(x_bf[:].flatten(), x_d[:].flatten())
        nc.gpsimd.dma_start(wk_bf[:].flatten(), wk_d[:].flatten())
        nc.gpsimd.dma_start(wv_bf[:].flatten(), wv_d[:].flatten())
        nc.gpsimd.dma_start(wq_bf[:].flatten(), wq_d[:].flatten())
        nc.gpsimd.dma_start(wo_bf[:].flatten(), wo_d[:].flatten())

        # ---------- persistent activations ----------
        qkv = est.enter_context(tc.tile_pool(name="qkv", bufs=1))
        qt = [qkv.tile([P, N], BF16, name=f"qt{c}", tag=f"qt{c}") for c in range(IC)]
        kt = [qkv.tile([P, J], BF16, name=f"kt{c}", tag=f"kt{c}") for c in range(IC)]
        # v padded: per head 64 cols of V then a ones column (65 per head)
        vp = [qkv.tile([P, H * 65], BF16, name=f"vp{c}", tag=f"vp{c}")
              for c in range(JC)]

        o_bf = est.enter_context(tc.tile_pool(name="o_bf", bufs=1))
        # ot[ic] [P, N]: normalized attention output, transposed layout
        # [INNER, N]; head h lives in chunk h//2, partitions (h%2)*64.
        ot = [o_bf.tile([P, N], BF16, name=f"ot{c}", tag=f"ot{c}")
              for c in range(IC)]

        wo_pool = est.enter_context(tc.tile_pool(name="wo_pool", bufs=1))
        wo_sb = wo_pool.tile([P, IC * QD], BF16, name="wo_sb")
        nc.sync.dma_start(
            wo_sb[:].rearrange("p (c n) -> p c n", c=IC),
            wo_bf[:].rearrange("(c p) n -> p c n", p=P))

        # ---------- phase A: ctx path (wk, wv, ctxT, KT, Vpad) ----------
        with ExitStack() as actx:
            wkv = actx.enter_context(tc.tile_pool(name="wkv", bufs=1))
            wk_sb = wkv.tile([P, CC * INNER], BF16, name="wk_sb")
            wv_sb = wkv.tile([P, CC * INNER], BF16, name="wv_sb")
            nc.sync.dma_start(
                wk_sb[:].rearrange("p (c n) -> p c n", c=CC),
                wk_bf[:].rearrange("(c p) n -> p c n", p=P))
            nc.sync.dma_start(
                wv_sb[:].rearrange("p (c n) -> p c n", c=CC),
                wv_bf[:].rearrange("(c p) n -> p c n", p=P))

            ctxT_p = actx.enter_context(tc.tile_pool(name="ctxT_p", bufs=1))
            ctxT = [ctxT_p.tile([P, J], BF16, name=f"ctxT{c}", tag=f"ctxT{c}")
                    for c in range(CC)]

            ppsum = actx.enter_context(
                tc.tile_pool(name="ppsum", bufs=4, space="PSUM"))

            for cc in range(CC):
                nc.sync.dma_start(
                    ctxT[cc][:], ctx_bf[:, cc * P:(cc + 1) * P], transpose=True)

            # KT[ic] [P, J]: stationary Wk chunk, moving ctxT
            for ic in range(IC):
                for jb in range(J // NBW):
                    kp = ppsum.tile([P, NBW], F32, name="kp", tag="pp")
                    for cc in range(CC):
                        nc.tensor.matmul(
                            kp[:],
                            wk_sb[:, cc * INNER + ic * P: cc * INNER + (ic + 1) * P],
                            ctxT[cc][:, jb * NBW:(jb + 1) * NBW],
                            start=(cc == 0), stop=(cc == CC - 1),
                        )
                    nc.vector.tensor_copy(
                        kt[ic][:, jb * NBW:(jb + 1) * NBW], kp[:])

            # V natural [J, INNER] -> padded per head (65 cols per head)
            for jc in range(JC):
                for vb in range(INNER // NBW):
                    vpp = ppsum.tile([P, NBW], F32, name="vpp", tag="pp")
                    for cc in range(CC):
                        nc.tensor.matmul(
                            vpp[:],
                            ctxT[cc][:, jc * P:(jc + 1) * P],
                            wv_sb[:, cc * INNER + vb * NBW: cc * INNER + (vb + 1) * NBW],
                            start=(cc == 0), stop=(cc == CC - 1),
                        )
                    hpb = NBW // Dh  # heads per block = 8
                    dst = vp[jc][:, vb * hpb * 65:(vb + 1) * hpb * 65]
                    dst = dst.rearrange("p (h e) -> p h e", e=65)[:, :, 0:64]
                    src = vpp[:].rearrange("p (h e) -> p h e", e=Dh)
                    nc.vector.tensor_copy(dst, src)
                ones_cols = vp[jc][:].rearrange(
                    "p (h e) -> p h e", e=65)[:, :, 64:65]
                nc.vector.memset(ones_cols, 1.0)

        # ---------- phase B: x path (wq, xT, QT) ----------
        with ExitStack() as bctx:
            wq_pool = bctx.enter_context(tc.tile_pool(name="wq_pool", bufs=1))
            wq_sb = wq_pool.tile([P, QC * INNER], BF16, name="wq_sb")
            nc.sync.dma_start(
                wq_sb[:].rearrange("p (c n) -> p c n", c=QC),
                wq_bf[:].rearrange("(c p) n -> p c n", p=P))

            xT_p = bctx.enter_context(tc.tile_pool(name="xT_p", bufs=1))
            xT = [xT_p.tile([P, N], BF16, name=f"xT{c}", tag=f"xT{c}")
                  for c in range(QC)]

            ppsum2 = bctx.enter_context(
                tc.tile_pool(name="ppsum2", bufs=4, space="PSUM"))

            for qc in range(QC):
                nc.sync.dma_start(
                    xT[qc][:], x_bf[:, qc * P:(qc + 1) * P], transpose=True)

            for ic in range(IC):
                for nb in range(NB):
                    qp = ppsum2.tile([P, NBW], F32, name="qp", tag="qp2")
                    for qc in range(QC):
                        nc.tensor.matmul(
                            qp[:],
                            wq_sb[:, qc * INNER + ic * P: qc * INNER + (ic + 1) * P],
                            xT[qc][:, nb * NBW:(nb + 1) * NBW],
                            start=(qc == 0), stop=(qc == QC - 1),
                        )
                    nc.vector.tensor_copy(
                        qt[ic][:, nb * NBW:(nb + 1) * NBW], qp[:])

        # ---------- phase C: attention ----------
        # PV in O^T orientation: stationary Vpad [128, 65], moving P^T
        # [128, 512] -> psum [65, 512]; row 64 is the softmax denominator.
        # Normalize: denom row -> den_all (ACT), PE ones-broadcast to 64
        # partitions, DVE reciprocal, DVE multiply psum*recip -> ot (bf16).
        with ExitStack() as cctx:
            pt_pool = cctx.enter_context(tc.tile_pool(name="pt_pool", bufs=2))
            rd_pool = cctx.enter_context(tc.tile_pool(name="rd_pool", bufs=2))
            den_pool = cctx.enter_context(tc.tile_pool(name="den_pool", bufs=4))
            spsum = cctx.enter_context(
                tc.tile_pool(name="spsum", bufs=2, space="PSUM"))
            pvpsum = cctx.enter_context(
                tc.tile_pool(name="pvpsum", bufs=2, space="PSUM"))
            bcpsum = cctx.enter_context(
                tc.tile_pool(name="bcpsum", bufs=2, space="PSUM"))

            for h in range(H):
                ic = h // 2
                po = (h % 2) * Dh
                pts = []
                for jc in range(JC):
                    ptile = pt_pool.tile([P, N], BF16, name=f"pt{jc}",
                                         tag=f"pt{jc}")
                    pts.append(ptile)
                    for half in range(2):
                        sp = spsum.tile([P, 2 * NBW], F32, name="sp", tag="sp")
                        for nbh in range(2):
                            nb = half * 2 + nbh
                            nc.tensor.matmul(
                                sp[:, nbh * NBW:(nbh + 1) * NBW],
                                kt[ic][po:po + Dh, jc * P:(jc + 1) * P],
                                qt[ic][po:po + Dh, nb * NBW:(nb + 1) * NBW],
                                start=True, stop=True,
                            )
                        nc.scalar.activation(
                            ptile[:, half * 2 * NBW:(half + 1) * 2 * NBW],
                            sp[:], EXP, scale=SCALE)

                for nb in range(NB):
                    pv = pvpsum.tile([65, NBW], F32, name="pv", tag="pv")
                    for jc in range(JC):
                        nc.tensor.matmul(
                            pv[:],
                            vp[jc][:, h * 65: h * 65 + 65],
                            pts[jc][:, nb * NBW:(nb + 1) * NBW],
                            start=(jc == 0), stop=(jc == JC - 1),
                        )
                    # denominator row -> bf16 (ACT), broadcast to 64
                    # partitions via K=1 matmul, reciprocal, multiply.
                    den_t = den_pool.tile([1, NBW], BF16, name="den_t",
                                          tag="den_t")
                    nc.vector.tensor_copy(den_t[:], pv[64:65, :])
                    bc = bcpsum.tile([Dh, NBW], F32, name="bc", tag="bc")
                    nc.tensor.matmul(
                        bc[:], ones_bf[:, 0:Dh], den_t[:],
                        start=True, stop=True)
                    rden = rd_pool.tile([Dh, NBW], F32, name="rden", tag="rden")
                    nc.vector.reciprocal(rden[:], bc[:])
                    nc.vector.tensor_tensor(
                        ot[ic][po:po + Dh, nb * NBW:(nb + 1) * NBW],
                        pv[0:Dh, :], rden[:], op=mybir.AluOpType.mult)

        # ---------- phase D: output projection ----------
        with ExitStack() as dctx:
            opsum = dctx.enter_context(
                tc.tile_pool(name="opsum", bufs=4, space="PSUM"))
            ostage_p = dctx.enter_context(tc.tile_pool(name="ostage_p", bufs=4))

            for nt in range(NT):
                for qb in range(QD // NBW):
                    op = opsum.tile([P, NBW], F32, name="op", tag="op")
                    for ic in range(IC):
                        nc.tensor.matmul(
                            op[:],
                            ot[ic][:, nt * P:(nt + 1) * P],
                            wo_sb[:, ic * QD + qb * NBW: ic * QD + (qb + 1) * NBW],
                            start=(ic == 0), stop=(ic == IC - 1),
                        )
                    ostage = ostage_p.tile([P, NBW], F32, name="ostage",
                                           tag="ostage")
                    nc.vector.tensor_tensor(
                        ostage[:], op[:], bias_bc[:, qb * NBW:(qb + 1) * NBW],
                        op=mybir.AluOpType.add)
                    nc.sync.dma_start(
                        out_d[nt * P:(nt + 1) * P, qb * NBW:(qb + 1) * NBW],
                        ostage[:])


def _get_module():
    if "nc" not in _CACHE:
        _CACHE["nc"] = _build_module()
    return _CACHE["nc"]


def kernel(x, context, Wq, Wk, Wv, Wo, bo):
    nc = _get_module()
    x = np.asarray(x, dtype=np.float32)
    context = np.asarray(context, dtype=np.float32)
    Wq = np.asarray(Wq, dtype=np.float32)
    Wk = np.asarray(Wk, dtype=np.float32)
    Wv = np.asarray(Wv, dtype=np.float32)
    Wo = np.asarray(Wo, dtype=np.float32)
    bo = np.asarray(bo, dtype=np.float32)

    in_maps = [
        {
            "x": np.ascontiguousarray(x[b]),
            "context": np.ascontiguousarray(context[b]),
            "Wq": Wq, "Wk": Wk, "Wv": Wv, "Wo": Wo, "bo": bo,
        }
        for b in range(B)
    ]
    res = bass_utils.run_bass_kernel_spmd(nc, in_maps, core_ids=list(range(B)))
    return np.stack([res.results[b]["out"] for b in range(B)], axis=0)


if __name__ == "__main__":
    nc = _get_module()
    print("module built and compiled OK")



# revision 25
# speedup vs baseline: 1.3805x; 1.3805x over previous
"""Trainium2 Bass kernel for nn_CrossAttentionLayer (batch-parallel, 8 cores).

Per-core computation (one batch element):
    q = x @ Wq      [N, INNER]      k = ctx @ Wk    [J, INNER]
    v = ctx @ Wv    [J, INNER]
    per head h: S^T = K_h^T-contract-d Q_h ; P^T = exp(scale*S^T)
    O[n, d] = sum_j P[n,j] V[j,d]  (natural orientation; ones column in V
    makes psum col 64 the softmax denominator); O /= den
    out = O @ Wo + bo

Engine plan (cost-model-driven):
  - matmul cost ~ out_free_size, contraction-independent -> PV in natural
    orientation ([128 n, 65] psum) halves its PE cost vs O^T orientation
    and yields the denominator for free.
  - exp on ACT (~266us) is co-critical with PE (~320us): one interleaved
    emission schedule keeps ACT streaming while PE does projections, S,
    PV and most of the output projection concurrently.
  - O^T comes from per-head-group DMA XBAR transposes (off the PE).
  - Output projection split into 3 head groups (3/3/2 inner chunks)
    accumulated into out_d via gpsimd DMA stores (group 0 plain+bias,
    groups 1-2 accum adds; single SWDGE queue preserves ordering), so
    only the last quarter of the projection serializes after the final
    exp.
"""

import sys

if "/opt/trn_rl_repo" not in sys.path:
    sys.path.insert(0, "/opt/trn_rl_repo")

import numpy as np

import concourse.bass as bass
import concourse.mybir as mybir
import concourse.bacc as bacc
import concourse.tile as tile
from concourse import bass_utils
from concourse.masks import make_identity

P = 128
B, N, J = 8, 2048, 1024
QD, CD, H, Dh = 1024, 768, 16, 64
INNER = H * Dh
NT = N // P      # 16 n tiles
JC = J // P      # 8 context chunks
QC = QD // P     # 8 x-feature chunks
CC = CD // P     # 6 ctx-feature chunks
IC = INNER // P  # 8 inner chunks
NBW = 512        # moving-operand block width
NB = N // NBW    # 4
NH = 2           # n halves for the attention stream
NHW = N // NH    # 1024
NTH = NT // NH   # 8 n tiles per half
SCALE = float(Dh) ** -0.5

# output-projection head groups: inner chunks [0-2], [3-5], [6-7]
GBASE = [0, 3, 6]
GSIZE = [3, 3, 2]
GOF_H = [0, 6, 12]          # first head of each group
GRP_OF_H = [0] * 6 + [1] * 6 + [2] * 4

F32 = mybir.dt.float32
BF16 = mybir.dt.bfloat16
EXP = mybir.ActivationFunctionType.Exp
MUL = mybir.AluOpType.mult
ADD = mybir.AluOpType.add

_CACHE = {}


def _build_module():
    nc = bacc.Bacc("TRN2", target_bir_lowering=False, debug=False)

    x_d = nc.dram_tensor("x", [N, QD], F32, kind="ExternalInput")
    ctx_d = nc.dram_tensor("context", [J, CD], F32, kind="ExternalInput")
    wq_d = nc.dram_tensor("Wq", [QD, INNER], F32, kind="ExternalInput")
    wk_d = nc.dram_tensor("Wk", [CD, INNER], F32, kind="ExternalInput")
    wv_d = nc.dram_tensor("Wv", [CD, INNER], F32, kind="ExternalInput")
    wo_d = nc.dram_tensor("Wo", [INNER, QD], F32, kind="ExternalInput")
    bo_d = nc.dram_tensor("bo", [QD], F32, kind="ExternalInput")
    out_d = nc.dram_tensor("out", [N, QD], F32, kind="ExternalOutput")

    with tile.TileContext(nc) as tc:
        _emit(nc, tc, x_d, ctx_d, wq_d, wk_d, wv_d, wo_d, bo_d, out_d)

    nc.compile()
    return nc


def _emit(nc, tc, x_d, ctx_d, wq_d, wk_d, wv_d, wo_d, bo_d, out_d):
    from contextlib import ExitStack

    est = ExitStack()
    with est:
        # ---------- persistent pools (outer scope) ----------
        const = est.enter_context(tc.tile_pool(name="const", bufs=1))
        ident = const.tile([P, P], BF16, name="ident")
        ones_row = const.tile([1, P], BF16, name="ones_row")
        bias_bc = const.tile([P, QD], BF16, name="bias_bc")

        qkv = est.enter_context(tc.tile_pool(name="qkv", bufs=1))
        vp = [qkv.tile([P, H * 65], BF16, name=f"vp{c}", tag=f"vp{c}")
              for c in range(JC)]
        wo_sb = qkv.tile([P, IC * QD], BF16, name="wo_sb")

        wkq_pool = est.enter_context(tc.tile_pool(name="wkq_pool", bufs=1))
        wk_sb = wkq_pool.tile([P, CC * INNER], BF16, name="wk_sb")
        wq_sb = wkq_pool.tile([P, QC * INNER], BF16, name="wq_sb")

        KT_ROT = 4
        kt_pool = est.enter_context(tc.tile_pool(name="kt_pool", bufs=1))
        kt_tiles = {}

        def kt(ic):
            return kt_tiles[ic % KT_ROT]

        gp = est.enter_context(tc.tile_pool(name="gp", bufs=2, space="PSUM"))
        rd_pool = est.enter_context(tc.tile_pool(name="rd_pool", bufs=8))
        ostage_p = est.enter_context(tc.tile_pool(name="ostage_p", bufs=3))

        # ---------- scoped: alive through the stream ----------
        mid = ExitStack()
        est.enter_context(_closer(mid))
        xT_pool = mid.enter_context(tc.tile_pool(name="xT_pool", bufs=1))
        xT = [xT_pool.tile([P, N], BF16, name=f"xT{c}", tag=f"xT{c}")
              for c in range(QC)]
        ctxT_pool = mid.enter_context(tc.tile_pool(name="ctxT_pool", bufs=1))
        ctxT = [ctxT_pool.tile([P, J], BF16, name=f"ctxT{c}", tag=f"ctxT{c}")
                for c in range(CC)]
        wv_pool = mid.enter_context(tc.tile_pool(name="wv_pool", bufs=1))
        wv_sb = wv_pool.tile([P, CC * INNER], BF16, name="wv_sb")

        # ---------- scoped: preamble only ----------
        pre = ExitStack()
        xn_pool = pre.enter_context(tc.tile_pool(name="xn_pool", bufs=1))
        x_nat = xn_pool.tile([P, NT * QD], BF16, name="x_nat")
        ctx_pool = pre.enter_context(tc.tile_pool(name="ctx_pool", bufs=1))
        ctx_sb = ctx_pool.tile([P, JC * CD], BF16, name="ctx_sb")
        bo_sb = ctx_pool.tile([1, QD], BF16, name="bo_sb")
        tp = pre.enter_context(tc.tile_pool(name="tp", bufs=4, space="PSUM"))

        # ---------- DMA issue order (gpsimd queue, serialized) ----------
        make_identity(nc, ident[:])
        nc.gpsimd.memset(ones_row[:], 1.0)
        x_nat_v = x_nat[:].rearrange("p (c n) -> p c n", c=NT)
        x_d_v = x_d[:].rearrange("(c p) n -> p c n", p=P)
        nc.gpsimd.dma_start(x_nat_v[:, 0:NTH, :], x_d_v[:, 0:NTH, :])
        nc.gpsimd.dma_start(
            ctx_sb[:].rearrange("p (c n) -> p c n", c=JC),
            ctx_d[:].rearrange("(c p) n -> p c n", p=P))
        nc.gpsimd.dma_start(
            wk_sb[:].rearrange("p (c n) -> p c n", c=CC),
            wk_d[:].rearrange("(c p) n -> p c n", p=P))
        nc.gpsimd.dma_start(x_nat_v[:, NTH:NT, :], x_d_v[:, NTH:NT, :])
        nc.gpsimd.dma_start(
            wq_sb[:].rearrange("p (c n) -> p c n", c=QC),
            wq_d[:].rearrange("(c p) n -> p c n", p=P))
        nc.gpsimd.dma_start(bo_sb[:], bo_d[:].unsqueeze(0))
        nc.gpsimd.dma_start(
            wv_sb[:].rearrange("p (c n) -> p c n", c=CC),
            wv_d[:].rearrange("(c p) n -> p c n", p=P))
        nc.gpsimd.dma_start(
            wo_sb[:].rearrange("p (c n) -> p c n", c=IC),
            wo_d[:].rearrange("(c p) n -> p c n", p=P))

        # ================= work-unit emitters =============================
        def emit_xT(nt_lo, nt_hi):
            # PE transposes x_nat [128, nt, 1024] -> xT[qc][:, nt*128..]
            for nt in range(nt_lo, nt_hi):
                for qc in range(QC):
                    t = tp.tile([P, J], BF16, name="tpt", tag="tp")
                    nc.tensor.transpose(
                        t[:, 0:P],
                        x_nat[:, nt * QD + qc * P: nt * QD + (qc + 1) * P],
                        ident[:])
                    dst = xT[qc][:, nt * P:(nt + 1) * P]
                    if nt < 4:
                        nc.scalar.copy(dst, t[:, 0:P])
                    else:
                        nc.vector.tensor_copy(dst, t[:, 0:P])

        def emit_ctxT():
            for jc in range(JC):
                for cc in range(CC):
                    t = tp.tile([P, J], BF16, name="tpt", tag="tp")
                    nc.tensor.transpose(
                        t[:, 0:P],
                        ctx_sb[:, jc * CD + cc * P: jc * CD + (cc + 1) * P],
                        ident[:])
                    dst = ctxT[cc][:, jc * P:(jc + 1) * P]
                    if (jc + cc) % 2 == 0:
                        nc.vector.tensor_copy(dst, t[:, 0:P])
                    else:
                        nc.scalar.copy(dst, t[:, 0:P])

        def emit_bias_bc():
            for qb in range(QD // NBW):
                bp = gp.tile([P, NBW], F32, name="bp", tag="gp")
                nc.tensor.matmul(
                    bp[:], ones_row[:, :], bo_sb[:, qb * NBW:(qb + 1) * NBW],
                    start=True, stop=True)
                nc.vector.tensor_copy(bias_bc[:, qb * NBW:(qb + 1) * NBW], bp[:])

        def emit_kproj(ic):
            kt_tiles[ic % KT_ROT] = kt_pool.tile(
                [P, J], BF16, name=f"kt{ic % KT_ROT}", tag=f"kt{ic % KT_ROT}")
            for jb in range(J // NBW):
                kp = gp.tile([P, NBW], F32, name="kp", tag="gp")
                for cc in range(CC):
                    nc.tensor.matmul(
                        kp[:],
                        wk_sb[:, cc * INNER + ic * P: cc * INNER + (ic + 1) * P],
                        ctxT[cc][:, jb * NBW:(jb + 1) * NBW],
                        start=(cc == 0), stop=(cc == CC - 1))
                nc.vector.tensor_copy(kt(ic)[:, jb * NBW:(jb + 1) * NBW], kp[:])

        def emit_vproj(jc):
            for vb in range(INNER // NBW):
                vpp = gp.tile([P, NBW], F32, name="vpp", tag="gp")
                for cc in range(CC):
                    nc.tensor.matmul(
                        vpp[:],
                        ctxT[cc][:, jc * P:(jc + 1) * P],
                        wv_sb[:, cc * INNER + vb * NBW: cc * INNER + (vb + 1) * NBW],
                        start=(cc == 0), stop=(cc == CC - 1))
                hpb = NBW // Dh  # 8 heads per block
                dst = vp[jc][:, vb * hpb * 65:(vb + 1) * hpb * 65]
                dst = dst.rearrange("p (h e) -> p h e", e=65)[:, :, 0:64]
                src = vpp[:].rearrange("p (h e) -> p h e", e=Dh)
                nc.vector.tensor_copy(dst, src)
            ones_cols = vp[jc][:].rearrange("p (h e) -> p h e", e=65)[:, :, 64:65]
            nc.vector.memset(ones_cols, 1.0)

        QT_ROT = 3
        qt_pool = None
        qt_tiles = {}

        def qt(ic):
            return qt_tiles[ic % QT_ROT]

        def emit_qproj_alloc(ic):
            qt_tiles[ic % QT_ROT] = qt_pool.tile(
                [P, N], BF16, name=f"qt{ic % QT_ROT}", tag=f"qt{ic % QT_ROT}")

        def emit_qproj_nb(ic, nb):
            qp = gp.tile([P, NBW], F32, name="qp", tag="gp")
            for qc in range(QC):
                nc.tensor.matmul(
                    qp[:],
                    wq_sb[:, qc * INNER + ic * P: qc * INNER + (ic + 1) * P],
                    xT[qc][:, nb * NBW:(nb + 1) * NBW],
                    start=(qc == 0), stop=(qc == QC - 1))
            nc.vector.tensor_copy(qt(ic)[:, nb * NBW:(nb + 1) * NBW], qp[:])

        # attention stream state
        pts_cur = {}    # (h, nh) -> list of 8 pts tiles
        on_tiles = {}   # nt -> o_nat tile [128, 384] for the current group
        oT_cur = [None]
        pt_pool = None
        on_pool = None
        oT_pool = None
        sp_pool = None
        pv_pool = None

        def emit_s_chunk(h, nh, jc):
            ic, po = h // 2, (h % 2) * Dh
            sp = sp_pool.tile([P, NHW], F32, name="sp", tag="sp")
            for s in range(NHW // NBW):
                nc.tensor.matmul(
                    sp[:, s * NBW:(s + 1) * NBW],
                    kt(ic)[po:po + Dh, jc * P:(jc + 1) * P],
                    qt(ic)[po:po + Dh, nh * NHW + s * NBW: nh * NHW + (s + 1) * NBW],
                    start=True, stop=True)
            ptile = pt_pool.tile([P, NHW], BF16, name=f"pt{jc}", tag=f"pt{jc}")
            pts_cur[(h, nh)].append(ptile)
            nc.scalar.activation(ptile[:], sp[:], EXP, scale=SCALE)

        def emit_pv(h, nh, ntl):
            nt = nh * NTH + ntl
            pts = pts_cur[(h, nh)]
            pv = pv_pool.tile([P, NBW], F32, name="pv", tag="pv")
            for jc in range(JC):
                nc.tensor.matmul(
                    pv[:, 0:65],
                    pts[jc][:, ntl * P:(ntl + 1) * P],
                    vp[jc][:, h * 65:(h + 1) * 65],
                    start=(jc == 0), stop=(jc == JC - 1))
            rden = rd_pool.tile([P, 1], F32, name="rden", tag="rden")
            nc.vector.reciprocal(rden[:], pv[:, 64:65])
            g = GRP_OF_H[h]
            hl = h - GOF_H[g]
            on = on_tiles[nt]
            nc.vector.tensor_scalar(
                on[:, hl * Dh:(hl + 1) * Dh], pv[:, 0:Dh], rden[:],
                None, op0=MUL)

        def emit_on_alloc():
            for nt in range(NT):
                on_tiles[nt] = on_pool.tile(
                    [P, 3 * P], BF16, name=f"on{nt}", tag=f"on{nt}")

        def emit_oT_alloc():
            oT_cur[0] = oT_pool.tile([P, 3 * N], BF16, name="oT", tag="oT")

        def emit_group_transpose(g, nh):
            # o_nat [128 n, gs*128 i] -> oT [128 i, gs, 128 n]
            gs = GSIZE[g]
            dst = oT_cur[0][:].rearrange("p (c n) -> p c n", c=3)
            for ntl in range(NTH):
                nt = nh * NTH + ntl
                nc.sync.dma_start(
                    dst[:, 0:gs, nt * P:(nt + 1) * P],
                    on_tiles[nt][:, 0:gs * P], transpose=True)

        def emit_oproj(g, nt):
            # both qb halves -> one [128, 1024] store (gpsimd, in-order)
            ostage = ostage_p.tile([P, QD], F32, name="ostage", tag="ostage")
            oT = oT_cur[0][:].rearrange("p (c n) -> p c n", c=3)
            for qb in range(QD // NBW):
                # in the tail (g==2) PV is done: borrow its psum banks for
                # the second chain to double pipelining
                pool = pv_pool if (g == 2 and qb == 1) else gp
                tag = "pv" if (g == 2 and qb == 1) else "gp"
                op = pool.tile([P, NBW], F32, name="op", tag=tag)
                for l in range(GSIZE[g]):
                    ic = GBASE[g] + l
                    nc.tensor.matmul(
                        op[:],
                        oT[:, l, nt * P:(nt + 1) * P],
                        wo_sb[:, ic * QD + qb * NBW: ic * QD + (qb + 1) * NBW],
                        start=(l == 0), stop=(l == GSIZE[g] - 1))
                dst = ostage[:, qb * NBW:(qb + 1) * NBW]
                if g == 0:
                    nc.vector.tensor_tensor(
                        dst, op[:], bias_bc[:, qb * NBW:(qb + 1) * NBW], op=ADD)
                else:
                    nc.vector.tensor_copy(dst, op[:])
            if g == 0:
                nc.gpsimd.dma_start(out_d[nt * P:(nt + 1) * P, :], ostage[:])
            else:
                nc.gpsimd.dma_start(out_d[nt * P:(nt + 1) * P, :], ostage[:],
                                    accum_op=ADD)

        # ================= preamble PE work ================================
        emit_xT(0, NTH)
        emit_ctxT()
        emit_kproj(0)
        emit_xT(NTH, NT)
        emit_bias_bc()
        pre.close()

        qt_pool = est.enter_context(tc.tile_pool(name="qt_pool", bufs=1))
        pt_pool = est.enter_context(tc.tile_pool(name="pt_pool", bufs=2))
        on_pool = est.enter_context(tc.tile_pool(name="on_pool", bufs=1))
        oT_pool = est.enter_context(tc.tile_pool(name="oT_pool", bufs=1))
        sp_pool = est.enter_context(tc.tile_pool(name="spp", bufs=2, space="PSUM"))
        pv_pool = est.enter_context(tc.tile_pool(name="pvp", bufs=2, space="PSUM"))

        emit_qproj_alloc(0)
        emit_qproj_nb(0, 0)
        emit_qproj_nb(0, 1)

        # ================= fill-work schedule =============================
        n_chunks = NH * H  # 32
        fill_sched = {c: [] for c in range(n_chunks)}
        # Vproj fully emitted before PV(0,0) at the end of chunk 1
        fill_sched[0].append(lambda: emit_qproj_nb(0, 2))
        fill_sched[0].append(lambda: emit_qproj_nb(0, 3))
        for jc in range(4):
            fill_sched[0].append(lambda jc=jc: emit_vproj(jc))
        for jc in range(4, JC):
            fill_sched[1].append(lambda jc=jc: emit_vproj(jc))
        # Kproj(ic) before chunk 4ic (kt slot ic%4 free after chunk 4ic-13)
        for ic in range(1, IC):
            fill_sched[max(3, 4 * ic - 7)].append(
                lambda ic=ic: emit_kproj(ic))
        # Qproj(ic): nb0-1 before chunk 4ic, nb2-3 before chunk 4ic+1
        qsched = {1: [(2, "a"), (2, 0), (2, 1), (3, 2), (3, 3)],
                  2: [(4, "a"), (4, 0), (5, 1), (6, 2), (7, 3)]}
        for ic in range(3, IC):
            b = 4 * ic - 9
            qsched[ic] = [(b, "a"), (b, 0), (b + 1, 1), (b + 2, 2), (b + 3, 3)]
        for ic, entries in qsched.items():
            for c, u in entries:
                if u == "a":
                    fill_sched[c].append(lambda ic=ic: emit_qproj_alloc(ic))
                else:
                    fill_sched[c].append(
                        lambda ic=ic, nb=u: emit_qproj_nb(ic, nb))
        # Oproj: g0 over chunks 13-22 (transposes done c=11/12),
        #        g1 over chunks 25-31 (transposes done c=23/24), g2 in tail
        for i in range(NT):
            fill_sched[13 + (i * 10) // NT].append(
                lambda nt=i: emit_oproj(0, nt))
        for i in range(NT):
            fill_sched[25 + (i * 7) // NT].append(
                lambda nt=i: emit_oproj(1, nt))

        # ================= attention stream ===============================
        emit_oT_alloc()
        emit_on_alloc()

        for c in range(n_chunks):
            h, nh = c // 2, c % 2
            pts_cur[(h, nh)] = []
            for jc in range(JC):
                emit_s_chunk(h, nh, jc)
            for th in fill_sched[c]:
                th()
            if c >= 1:
                ph, pnh = (c - 1) // 2, (c - 1) % 2
                for ntl in range(NTH):
                    emit_pv(ph, pnh, ntl)
                if ph in (5, 11):
                    if ph == 11 and pnh == 0:
                        # fresh oT for group 1 (waits og0 readers, done c=22)
                        emit_oT_alloc()
                    emit_group_transpose(GRP_OF_H[ph], pnh)
                    if pnh == 1:
                        emit_on_alloc()
                del pts_cur[(ph, pnh)]

        # ================= tail ===========================================
        # chunk 31 already emitted PV(15, 0)
        emit_oT_alloc()
        emit_group_transpose(2, 0)
        for ntl in range(NTH):
            emit_pv(H - 1, 1, ntl)
        emit_group_transpose(2, 1)
        for nt in range(NT):
            emit_oproj(2, nt)


class _closer:
    """Adapter so an inner ExitStack is closed by the outer one (LIFO)."""

    def __init__(self, stack):
        self.stack = stack

    def __enter__(self):
        return self.stack

    def __exit__(self, *exc):
        return self.stack.__exit__(*exc)


def _get_module():
    if "nc" not in _CACHE:
        _CACHE["nc"] = _build_module()
    return _CACHE["nc"]


def kernel(x, context, Wq, Wk, Wv, Wo, bo):
    nc = _get_module()
    x = np.asarray(x, dtype=np.float32)
    context = np.asarray(context, dtype=np.float32)
    Wq = np.asarray(Wq, dtype=np.float32)
    Wk = np.asarray(Wk, dtype=np.float32)
    Wv = np.asarray(Wv, dtype=np.float32)
    Wo = np.asarray(Wo, dtype=np.float32)
    bo = np.asarray(bo, dtype=np.float32)

    in_maps = [
        {
            "x": np.ascontiguousarray(x[b]),
            "context": np.ascontiguousarray(context[b]),
            "Wq": Wq, "Wk": Wk, "Wv": Wv, "Wo": Wo, "bo": bo,
        }
        for b in range(B)
    ]
    res = bass_utils.run_bass_kernel_spmd(nc, in_maps, core_ids=list(range(B)))
    return np.stack([res.results[b]["out"] for b in range(B)], axis=0)


if __name__ == "__main__":
    nc = _get_module()
    print("module built and compiled OK")
